# revision 29
# baseline (speedup 1.0000x reference)
"""CRF loss (forward-algorithm denominator + gold-path numerator) on 8 Trainium2 cores.

Strategy (data-parallel over batch, 8 batch elements per core):
  The forward recursion alpha_t[j] = logsumexp_i(scores[t,i,j] + alpha_{t-1}[i])
  is run in LINEAR space:  v_t = E_t^T v_{t-1},  E_t = exp(scores[t] - KAPPA),
  with the constant normalizer KAPPA absorbing the ~log(T)+E[e^s] growth per
  step, so no per-step max/renormalization is needed (drift stays ~O(10) nats).
  denominator = log(v_S[END]) + S*KAPPA.

  On-chip layout per step: E-tile [128=(h,i), 256=(g,j)] where batch q = 4h+g.
  Per (t, g): one PE matmul  lhsT=E_g [128,64] (weights), rhs=vsel[:,2g:2g+2]
  (selector carrying v for the two batches of group g in its two partition
  halves) -> out vT[64=j, 2] in PSUM.  vT columns feed the next step's vsel
  via two strided DVE copies.  exp() runs as large per-chunk ACT instructions
  off the critical chain.

  numerator: indirect-DMA row gather of scores[t,b,ti,:], multiply by a
  host-built (j==tj)*mask one-hot mask, free-axis reduce, and a final
  selector matmul for the per-batch cross-partition sums.
"""
import math
import numpy as np

S = 512
B = 64
T = 64
BQ = 8          # batch per core
N_CORES = 8
START_TAG = 62
END_TAG = 63
T_CHUNK = 8     # time steps per DMA/exp super-tile
N_CHUNKS = S // T_CHUNK
KAPPA = float(np.float32(math.log(T) + 0.5))

_COMPILED = None


def _build(n_chunks=N_CHUNKS, with_numer=True, repeat=1):
    import concourse.bass as bass
    import concourse.bacc as bacc
    import concourse.mybir as mybir
    import concourse.tile as tile
    from concourse._compat import axon_active

    dt = mybir.dt
    AF = mybir.ActivationFunctionType
    ALU = mybir.AluOpType

    nc = bacc.Bacc(
        "TRN2", target_bir_lowering=False, debug=not axon_active(), num_devices=N_CORES
    )

    scores = nc.declare_dram_parameter("scores", [S, BQ, T, T], dt.bfloat16, isOutput=False)
    # host-prepared constants / index tensors
    vinit_d = nc.declare_dram_parameter("vinit", [128, 8], dt.float32, isOutput=False)
    sel8_d = nc.declare_dram_parameter("sel8", [128, 8], dt.float32, isOutput=False)
    oh63_d = nc.declare_dram_parameter("oh63", [64, 1], dt.float32, isOutput=False)
    offs_d = nc.declare_dram_parameter("offs", [128, 32], dt.int32, isOutput=False)
    eqm_d = nc.declare_dram_parameter("eqmask", [128, 32 * 64], dt.float32, isOutput=False)
    loss_d = nc.declare_dram_parameter("loss", [BQ, 1], dt.float32, isOutput=True)

    with tile.TileContext(nc) as tc:
        with (
            tc.tile_pool(name="static", bufs=1) as static_pool,
            tc.tile_pool(name="ering", bufs=4) as ering,
            tc.tile_pool(name="vt", bufs=2, space="PSUM") as vt_pool,
            tc.tile_pool(name="fin", bufs=1, space="PSUM") as fin_psum,
            tc.tile_pool(name="fins", bufs=1) as fin_sbuf,
        ):
            # ---- static tiles ----
            vselA = static_pool.tile([128, 8], dt.float32)
            vselB = static_pool.tile([128, 8], dt.float32)
            sel8 = static_pool.tile([128, 8], dt.float32)
            oh63 = static_pool.tile([64, 1], dt.float32)
            kbias = static_pool.tile([128, 1], dt.float32)
            nc.vector.memset(kbias[:], -KAPPA)
            zbias = static_pool.tile([128, 1], dt.float32)
            nc.vector.memset(zbias[:], 0.0)

            nc.sync.dma_start(out=vselA[:], in_=vinit_d[:])
            nc.sync.dma_start(out=sel8[:], in_=sel8_d[:])
            nc.sync.dma_start(out=oh63[:], in_=oh63_d[:])
            nc.vector.memset(vselB[:], 0.0)
            if with_numer:
                offs = static_pool.tile([128, 32], dt.int32)
                eqm = static_pool.tile([128, 32 * 64], dt.float32)
                gath = static_pool.tile([128, 32 * 64], dt.bfloat16)
                gathf = static_pool.tile([128, 32 * 64], dt.float32)
                prod = static_pool.tile([128, 32 * 64], dt.float32)
                npart = static_pool.tile([128, 1], dt.float32)
                nc.sync.dma_start(out=offs[:], in_=offs_d[:])
                nc.sync.dma_start(out=eqm[:], in_=eqm_d[:])

            # ---- numerator row gather (background, gpsimd queue) ----
            if with_numer:
                rows = scores[:].rearrange("t q i j -> (t q i) j")
                g3 = gath[:].rearrange("p (n j) -> p n j", n=32)
                for n in range(32):
                    nc.gpsimd.indirect_dma_start(
                        out=g3[:, n, :],
                        out_offset=None,
                        in_=rows,
                        in_offset=bass.IndirectOffsetOnAxis(
                            ap=offs[:, n : n + 1], axis=0
                        ),
                    )

            # ---- main scan ----
            vsel_cur = vselA
            vsel_nxt = vselB
            vt_last = None
            for c in [cc for _ in range(repeat) for cc in range(n_chunks)]:
                et = ering.tile([128, T_CHUNK * 256], dt.bfloat16, tag="et")
                etf = ering.tile([128, T_CHUNK * 256], dt.float32, tag="etf")
                eb4 = et[:].rearrange("p (t g j) -> p t g j", t=T_CHUNK, g=4)
                e4 = etf[:].rearrange("p (t g j) -> p t g j", t=T_CHUNK, g=4)
                # one DMA per batch slot q = 4h+g: dst partitions h-half, free (t, g, j)
                src = scores[c * T_CHUNK : (c + 1) * T_CHUNK]
                for q in range(BQ):
                    h, g = q // 4, q % 4
                    nc.sync.dma_start(
                        out=eb4[64 * h : 64 * h + 64, :, g, :],
                        in_=src[:, q].rearrange("t i j -> i t j"),
                    )
                nc.scalar.activation(out=etf[:], in_=et[:], func=AF.Exp, bias=kbias[:])

                for τ in range(T_CHUNK):
                    vt = vt_pool.tile([64, 8], dt.float32, tag="vt", space="PSUM")
                    for g in range(4):
                        nc.tensor.matmul(
                            out=vt[:, 2 * g : 2 * g + 2],
                            lhsT=e4[:, τ, g, :],
                            rhs=vsel_cur[:, 2 * g : 2 * g + 2],
                            start=True,
                            stop=True,
                        )
                    # build next selector: vsel[0:64, even] <- vT even cols,
                    # vsel[64:128, odd] <- vT odd cols
                    v2 = vt[:].rearrange("j (g c) -> j g c", c=2)
                    nc.vector.tensor_copy(
                        out=vsel_nxt[0:64].rearrange("i (g c) -> i g c", c=2)[:, :, 0],
                        in_=v2[:, :, 0],
                    )
                    nc.vector.tensor_copy(
                        out=vsel_nxt[64:128].rearrange("i (g c) -> i g c", c=2)[:, :, 1],
                        in_=v2[:, :, 1],
                    )
                    vsel_cur, vsel_nxt = vsel_nxt, vsel_cur
                    vt_last = vt

            # ---- numerator reduction ----
            numer = fin_psum.tile([8, 1], dt.float32, space="PSUM")
            if with_numer:
                nc.vector.tensor_copy(out=gathf[:], in_=gath[:])
                nc.vector.tensor_tensor(
                    out=prod[:], in0=gathf[:], in1=eqm[:], op=ALU.mult
                )
                nc.vector.tensor_reduce(
                    out=npart[:], in_=prod[:], axis=mybir.AxisListType.X, op=ALU.add
                )
                nc.tensor.matmul(
                    out=numer[:], lhsT=sel8[:], rhs=npart[:], start=True, stop=True
                )
            else:
                nc.tensor.matmul(
                    out=numer[:], lhsT=sel8[:, 0:8], rhs=zbias[:], start=True, stop=True
                )

            # ---- final assembly ----
            vlast_sb = fin_sbuf.tile([64, 8], dt.float32)
            nc.vector.tensor_copy(out=vlast_sb[:], in_=vt_last[:])
            dps = fin_psum.tile([8, 1], dt.float32, space="PSUM")
            nc.tensor.matmul(out=dps[:], lhsT=vlast_sb[:], rhs=oh63[:], start=True, stop=True)
            dlog = fin_sbuf.tile([8, 1], dt.float32)
            nc.scalar.activation(out=dlog[:], in_=dps[:], func=AF.Ln, bias=zbias[0:8])
            dmn = fin_sbuf.tile([8, 1], dt.float32)
            nc.vector.tensor_tensor(out=dmn[:], in0=dlog[:], in1=numer[:], op=ALU.subtract)
            lossv = fin_sbuf.tile([8, 1], dt.float32)
            nc.vector.tensor_scalar(
                out=lossv[:],
                in0=dmn[:],
                scalar1=float(n_chunks * T_CHUNK * KAPPA),
                scalar2=1.0 / B,
                op0=ALU.add,
                op1=ALU.mult,
            )
            nc.sync.dma_start(out=loss_d[:], in_=lossv[:])

    nc.compile()
    return nc


def _host_inputs(scores, target, mask):
    """Build per-core input maps. Device batch slot q for core c = original batch 8c+q."""
    import ml_dtypes

    scores = np.ascontiguousarray(scores, dtype=np.float32).astype(ml_dtypes.bfloat16)
    target = np.asarray(target, dtype=np.int32)
    mask = np.asarray(mask, dtype=np.int32)

    # constants shared by all cores
    vinit = np.zeros((128, 8), dtype=np.float32)
    for col in range(8):
        h = col & 1
        vinit[h * 64 + START_TAG, col] = 1.0
    # numerator selector: partition block of device slot q sums into column
    # beta = 2*(q%4) + q//4 so numer is beta-indexed like the denominator
    sel8 = np.zeros((128, 8), dtype=np.float32)
    for q in range(8):
        beta = 2 * (q % 4) + q // 4
        sel8[q * 16 : q * 16 + 16, beta] = 1.0
    oh63 = np.zeros((64, 1), dtype=np.float32)
    oh63[END_TAG, 0] = 1.0

    ti = (target // T).astype(np.int64)  # (S, B)
    tj = (target % T).astype(np.int64)
    jr = np.arange(64)

    in_maps = []
    for c in range(N_CORES):
        bsl = slice(c * BQ, (c + 1) * BQ)
        sc = np.ascontiguousarray(scores[:, bsl])  # (S, 8, T, T)
        offs = np.zeros((128, 32), dtype=np.int32)
        eqmask = np.zeros((128, 32, 64), dtype=np.float32)
        for q in range(BQ):
            b = c * BQ + q
            for t in range(S):
                p = q * 16 + (t % 16)
                n = t // 16
                offs[p, n] = t * (BQ * T) + q * T + int(ti[t, b])
                eqmask[p, n] = (jr == tj[t, b]) * float(mask[t, b])
        in_maps.append(
            {
                "scores": sc,
                "vinit": vinit,
                "sel8": sel8,
                "oh63": oh63,
                "offs": offs,
                "eqmask": eqmask.reshape(128, 32 * 64),
            }
        )
    return in_maps


def kernel(scores, target, mask):
    global _COMPILED
    from concourse.bass_utils import run_bass_kernel_spmd

    if _COMPILED is None:
        _COMPILED = _build()
    nc = _COMPILED
    in_maps = _host_inputs(scores, target, mask)
    res = run_bass_kernel_spmd(nc, in_maps, list(range(N_CORES)))

    loss = np.zeros(B, dtype=np.float32)
    for c in range(N_CORES):
        out = res.results[c]["loss"].reshape(BQ)  # indexed by beta = 2g+h
        for beta in range(BQ):
            h, g = beta & 1, beta >> 1
            q = 4 * h + g
            loss[c * BQ + q] = out[beta]
    return loss
